# revision 1
# baseline (speedup 1.0000x reference)
"""Mixtral sparse MoE block on 8 Trainium2 NeuronCores.

Strategy (per sharding hint): expert parallelism. E=8 experts, 8 cores,
one expert per core. The router (gate matmul + top-2 + softmax) is tiny
(33 MFLOP vs 51.6 GFLOP of expert work) and data-dependent, so it runs
on host as part of the token dispatch: tokens are gathered per expert,
padded to a common length, each core runs its expert's SwiGLU MLP over
its tokens in bf16 (fp32 PSUM accumulation), and the host scatter-adds
the weighted per-expert outputs back (combine).

Device layout: features on partitions, tokens on the free dim.
  up[i,t]   = sum_h W1[h,i] * xT[h,t]   (lhsT = W1 tile, rhs = xT tile)
  gate[i,t] = sum_h W3[h,i] * xT[h,t]
  act[i,t]  = silu(up) * gate           (ACT silu + DVE mul, -> bf16)
  out[h,t]  = sum_i W2[i,h] * act[i,t]
No on-chip transposes needed anywhere.
"""

import numpy as np
import ml_dtypes

import bass_rust
import concourse.bass as bass
import concourse.mybir as mybir
import concourse.tile as tile
from concourse.bass_utils import run_bass_kernel_spmd
from concourse.tile import ScopedClock


def _enforce_single_wait(nc):
    """The walrus in this image rejects >1 sync-wait per instruction
    ("Too many sync wait commands", CoreV3GenImpl setupSyncWait). Hoist
    extra waits onto same-engine nops inserted just before the offender
    — waiting earlier on the same sequencer is always safe."""
    for f in nc.m.functions:
        for bb in f.blocks:
            insts = bb.instructions
            i = 0
            while i < len(insts):
                inst = insts[i]
                si = inst.sync_info
                if si is not None and len(si.on_wait) > 1:
                    waits = list(si.on_wait)
                    if any(w.wait_reg is not None for w in waits):
                        i += 1
                        continue
                    for j, w in enumerate(waits[:-1]):
                        nop = mybir.InstNoOp(
                            name=f"{inst.name}_hw{j}", ins=[], outs=[])
                        nop.engine = inst.engine
                        nop.sync_info = bass_rust.SyncInfo(
                            on_wait=[w], on_update=[])
                        insts.insert(i, nop)
                        i += 1
                    inst.sync_info = bass_rust.SyncInfo(
                        on_wait=[waits[-1]], on_update=list(si.on_update))
                i += 1

P = 128
H = 1024
I = 2048
E = 8
K = 2

BF16 = mybir.dt.bfloat16
F32 = mybir.dt.float32

# Populated by the last kernel() call so a harness can inspect HW timing.
LAST_RESULTS = None

_NC_CACHE = {}


def _t_chunks(t_pad):
    """Split the token free-dim into matmul chunks <= 512 (one PSUM bank)."""
    if t_pad <= 512:
        return [(0, t_pad)]
    half = (t_pad + 1) // 2
    half = (half + 31) // 32 * 32
    return [(0, half), (half, t_pad - half)]


def _build_nc(t_pad):
    """One expert's SwiGLU MLP over t_pad tokens (SPMD program, all cores)."""
    nc = bass.Bass()
    xT = nc.declare_dram_parameter("xT", [H, t_pad], BF16, isOutput=False)
    w1 = nc.declare_dram_parameter("w1", [H, I], BF16, isOutput=False)
    w3 = nc.declare_dram_parameter("w3", [H, I], BF16, isOutput=False)
    w2 = nc.declare_dram_parameter("w2", [I, H], BF16, isOutput=False)
    outT = nc.declare_dram_parameter("outT", [H, t_pad], F32, isOutput=True)

    HK = H // P    # 8 k-tiles over hidden dim
    IT = I // P    # 16 tiles over intermediate dim
    chunks = _t_chunks(t_pad)
    NW_CH = 4      # load W1/W3 in 4 column chunks of 512 so PE starts early
    WCH = I // NW_CH

    with tile.TileContext(nc) as tc:
        with (
            tc.tile_pool(name="x", bufs=1) as xpool,
            tc.tile_pool(name="wu", bufs=1) as wupool,
            tc.tile_pool(name="wg", bufs=1) as wgpool,
            tc.tile_pool(name="wd", bufs=1) as wdpool,
            tc.tile_pool(name="acts", bufs=1) as actpool,
            tc.tile_pool(name="ps", bufs=2, space="PSUM") as pspool,
            tc.tile_pool(name="ev", bufs=3) as evpool,
        ):
            x_sb = []
            for hk in range(HK):
                t = xpool.tile([P, t_pad], BF16, tag=f"x{hk}", name=f"x{hk}")
                nc.sync.dma_start(out=t[:], in_=xT[hk * P:(hk + 1) * P, :])
                x_sb.append(t)

            # Weights for up/gate, loaded in i-column chunks, chunk-major so
            # the first i-tiles' matmuls unblock after ~1/4 of W1+W3 arrives.
            w1_sb = [wupool.tile([P, I], BF16, tag=f"w1_{hk}", name=f"w1_{hk}")
                     for hk in range(HK)]
            w3_sb = [wgpool.tile([P, I], BF16, tag=f"w3_{hk}", name=f"w3_{hk}")
                     for hk in range(HK)]
            for c in range(NW_CH):
                cs = slice(c * WCH, (c + 1) * WCH)
                for hk in range(HK):
                    hs = slice(hk * P, (hk + 1) * P)
                    nc.sync.dma_start(out=w1_sb[hk][:, cs], in_=w1[hs, cs])
                    nc.sync.dma_start(out=w3_sb[hk][:, cs], in_=w3[hs, cs])

            w2_sb = []
            for it in range(IT):
                t = wdpool.tile([P, H], BF16, tag=f"w2_{it}", name=f"w2_{it}")
                nc.sync.dma_start(out=t[:], in_=w2[it * P:(it + 1) * P, :])
                w2_sb.append(t)

            act_sb = [actpool.tile([P, t_pad], BF16, tag=f"a{it}", name=f"a{it}")
                      for it in range(IT)]

            # PE warmup: dummy matmuls on the first x tile while the weight
            # DMAs stream in, so the HAM clock-gate is at 8/8 when the real
            # stream starts (~3.4us of activity needed).
            wn = min(384, t_pad)
            for wi in range(22):
                w_ps = pspool.tile([P, wn], F32, tag="warm", name=f"warm{wi}")
                nc.tensor.matmul(
                    w_ps[:], x_sb[0][:, 0:P], x_sb[0][:, 0:wn],
                    start=True, stop=True)

            # Phase A: up/gate matmuls + fused silu*gate eviction.
            for it in range(IT):
                isl = slice(it * P, (it + 1) * P)
                for (t0, tn) in chunks:
                    tsl = slice(t0, t0 + tn)
                    up_ps = pspool.tile([P, tn], F32, tag="up", name=f"up{it}_{t0}")
                    gt_ps = pspool.tile([P, tn], F32, tag="gt", name=f"gt{it}_{t0}")
                    for hk in range(HK):
                        nc.tensor.matmul(
                            up_ps[:], w1_sb[hk][:, isl], x_sb[hk][:, tsl],
                            start=(hk == 0), stop=(hk == HK - 1))
                    for hk in range(HK):
                        nc.tensor.matmul(
                            gt_ps[:], w3_sb[hk][:, isl], x_sb[hk][:, tsl],
                            start=(hk == 0), stop=(hk == HK - 1))
                    silu_t = evpool.tile([P, tn], F32, tag="silu", name=f"silu{it}_{t0}")
                    nc.scalar.activation(
                        silu_t[:], up_ps[:], mybir.ActivationFunctionType.Silu)
                    nc.vector.tensor_mul(act_sb[it][:, tsl], silu_t[:], gt_ps[:])

            # Phase B: down projection.
            for h in range(HK):
                hsl = slice(h * P, (h + 1) * P)
                for (t0, tn) in chunks:
                    tsl = slice(t0, t0 + tn)
                    o_ps = pspool.tile([P, tn], F32, tag="o", name=f"o{h}_{t0}")
                    for it in range(IT):
                        nc.tensor.matmul(
                            o_ps[:], w2_sb[it][:, hsl], act_sb[it][:, tsl],
                            start=(it == 0), stop=(it == IT - 1))
                    o_sb = evpool.tile([P, tn], F32, tag="osb", bufs=3,
                                       name=f"osb{h}_{t0}")
                    nc.scalar.copy(o_sb[:], o_ps[:])
                    nc.sync.dma_start(out=outT[hsl, tsl], in_=o_sb[:])

    _enforce_single_wait(nc)
    return nc


def kernel(x, Wg, W1, W2, W3, _trace=False):
    global LAST_RESULTS
    xf = np.asarray(x, dtype=np.float32).reshape(-1, H)
    T = xf.shape[0]

    # --- Host router: top-2 + softmax over the selected pair (fp32) ---
    logits = xf @ np.asarray(Wg, dtype=np.float32)           # (T, E)
    top2 = np.argsort(-logits, axis=-1)[:, :K]               # (T, K)
    v = np.take_along_axis(logits, top2, axis=-1)
    m = v.max(axis=-1, keepdims=True)
    p = np.exp(v - m)
    rw = (p / p.sum(axis=-1, keepdims=True)).astype(np.float32)

    # --- Dispatch: gather tokens per expert, pad to common length ---
    idx_e, wt_e = [], []
    for e in range(E):
        rows, slots = np.nonzero(top2 == e)
        idx_e.append(rows)
        wt_e.append(rw[rows, slots])
    cmax = max(len(r) for r in idx_e)
    t_pad = max(64, (cmax + 7) // 8 * 8)

    if t_pad not in _NC_CACHE:
        _NC_CACHE[t_pad] = _build_nc(t_pad)
    nc = _NC_CACHE[t_pad]

    in_maps = []
    for e in range(E):
        xT_e = np.zeros((H, t_pad), dtype=ml_dtypes.bfloat16)
        xT_e[:, :len(idx_e[e])] = xf[idx_e[e]].T.astype(ml_dtypes.bfloat16)
        in_maps.append({
            "xT": xT_e,
            "w1": np.asarray(W1[e], dtype=ml_dtypes.bfloat16),
            "w3": np.asarray(W3[e], dtype=ml_dtypes.bfloat16),
            "w2": np.asarray(W2[e], dtype=ml_dtypes.bfloat16),
        })

    res = run_bass_kernel_spmd(nc, in_maps, list(range(E)), trace=_trace)
    LAST_RESULTS = res

    # --- Combine: weighted scatter-add of per-expert outputs ---
    out = np.zeros((T, H), dtype=np.float32)
    for e in range(E):
        ne = len(idx_e[e])
        Ye = np.asarray(res.results[e]["outT"], dtype=np.float32)[:, :ne].T
        # rows are unique within one expert (top-2 indices are distinct)
        out[idx_e[e]] += Ye * wt_e[e][:, None]
    return out.reshape(np.asarray(x).shape).astype(np.float32)

